# revision 1
# baseline (speedup 1.0000x reference)
"""Multi-head attention (B=2, S=2048, D=1024, H=16) on 8 TRN2 NeuronCores.

Sharding: data-parallel over batch (2 groups of 4 cores) x head-parallel
(4 heads per core). W_q/W_k/W_v are column-sharded by head, W_o is
row-sharded; the 4 partial W_o outputs per batch are summed on the host
(the unshard step), which also undoes the device-side transposed layout.

Per-core kernel design: projection inputs (X, W_q/k/v) stream as bf16
(halves the HBM traffic); everything downstream - scores, probs, V, W_o -
is fp32-in-memory with float32r matmul inputs, which runs the PE at full
rate with ~1.5e-4 matmul error. End-to-end relative error ~4e-3.

  - Host pre-transposes X (Q/K/V inputs) and the weight slices so that
    every matmul contraction sits on the partition dim.
  - q/k projections produce qT/kT in [128 = 2 heads x 64 d, S] layout;
    the 1/sqrt(d_k) scale is folded into W_q/b_q on the host.
  - v projection produces v in natural [S, d] layout with a ones column
    appended per head, so the P@V matmul accumulates the softmax
    denominator (row 64 of the accumulator) for free.
  - scores are computed transposed ([k, sq] blocks); softmax skips the
    max-subtraction (scores are O(5) here, exp is safe in fp32), the
    denominator reciprocal is broadcast across partitions with a rank-1
    PE outer product.
  - causal structure: fully-masked [128 k x 512 sq] blocks are skipped,
    diagonal blocks are zeroed post-exp with gpsimd.affine_select. The
    block plan is derived from the actual mask input at call time, with
    a dense additive-mask fallback for non-causal patterns.
"""

import os

import numpy as np

_B, _S, _D, _H, _DK = 2, 2048, 1024, 16, 64
_HPC = 4          # heads per core
_NCORES = 8
_CPG = 4          # cores per (batch) group
_DPC = _HPC * _DK # 256 projection dims per core
_NEG = -1e9

_program_cache = {}
LAST_RESULTS = None  # BassKernelResults of the most recent run (for profiling)


def _analyze_mask(mask):
    """Classify each [128 k, 512 sq] block of mask^T. Returns (plan, dense).

    plan[i] = tuple of (j, mode, param) for sq-tile i; mode 0 = no mask,
    1 = causal affine_select (param = base), 2 = dense additive mask
    (param = index into dense blocks). Fully-masked blocks are omitted.
    """
    maskT = np.ascontiguousarray(mask.T)
    plan = []
    dense = []
    p_idx = np.arange(128)[:, None]
    s_idx = np.arange(512)[None, :]
    for i in range(_S // 512):
        row = []
        for j in range(_S // 128):
            blk = maskT[j * 128:(j + 1) * 128, i * 512:(i + 1) * 512]
            nz = blk != 0.0
            if nz.all():
                continue  # fully masked: block contributes nothing
            if not nz.any():
                row.append((j, 0, 0))
                continue
            base = i * 512 - j * 128
            causal = (s_idx + i * 512) < (p_idx + j * 128)
            if np.array_equal(nz, causal) and np.all(blk[nz] == 1.0):
                row.append((j, 1, base))
            else:
                row.append((j, 2, len(dense)))
                dense.append(blk * np.float32(_NEG))
        plan.append(tuple(row))
    if dense:
        dense_np = np.stack(dense).astype(np.float32)
    else:
        dense_np = np.zeros((1, 128, 512), np.float32)
    return tuple(plan), dense_np


def _build_program(plan, nblk):
    import concourse.bass as bass  # noqa: F401  (registers engine classes)
    import concourse.tile as tile
    from concourse import bacc, mybir

    F32 = mybir.dt.float32
    F32R = mybir.dt.float32r
    BF16 = mybir.dt.bfloat16
    AF = mybir.ActivationFunctionType
    ALU = mybir.AluOpType
    ts = bass.ts

    nc = bacc.Bacc(None, target_bir_lowering=False, debug=False)

    xq = nc.dram_tensor("xq", [_D, _S], BF16, kind="ExternalInput").ap()
    xk = nc.dram_tensor("xk", [_D, _S], BF16, kind="ExternalInput").ap()
    xv = nc.dram_tensor("xv", [_D, _S], BF16, kind="ExternalInput").ap()
    wq = nc.dram_tensor("wq", [_D, _DPC], BF16, kind="ExternalInput").ap()
    wk = nc.dram_tensor("wk", [_D, _DPC], BF16, kind="ExternalInput").ap()
    wv = nc.dram_tensor("wv", [_D, _DPC], BF16, kind="ExternalInput").ap()
    wo = nc.dram_tensor("wo", [_DPC, _D], F32R, kind="ExternalInput").ap()
    bq = nc.dram_tensor("bq", [_DPC], F32, kind="ExternalInput").ap()
    bk = nc.dram_tensor("bk", [_DPC], F32, kind="ExternalInput").ap()
    bvb = nc.dram_tensor("bvb", [128, _DPC], F32, kind="ExternalInput").ap()
    mblk = nc.dram_tensor("mblk", [nblk, 128, 512], F32, kind="ExternalInput").ap()
    y = nc.dram_tensor("y", [_D, _S], F32, kind="ExternalOutput").ap()

    with tile.TileContext(nc) as tc:
        from contextlib import ExitStack
        with ExitStack() as ctx:
            wpool = ctx.enter_context(tc.tile_pool(name="w", bufs=1))
            cpool = ctx.enter_context(tc.tile_pool(name="const", bufs=1))
            xcol_bufs = 6
            if any(m == 2 for row in plan for (_, m, _) in row) and nblk > 2:
                xcol_bufs = 5  # reclaim SBUF for the streamed mask tiles
            xpool = ctx.enter_context(tc.tile_pool(name="xcol", bufs=xcol_bufs))
            biga = ctx.enter_context(tc.tile_pool(name="biga", bufs=1))
            probp = ctx.enter_context(tc.tile_pool(name="probs", bufs=6))
            bcp = ctx.enter_context(tc.tile_pool(name="bc", bufs=6))
            recp = ctx.enter_context(tc.tile_pool(name="rec", bufs=4))
            yp = ctx.enter_context(tc.tile_pool(name="y", bufs=4))
            has_dense = any(m == 2 for row in plan for (_, m, _) in row)
            resident_mask = has_dense and nblk <= 2
            need_stream = has_dense and not resident_mask
            mpool = (
                ctx.enter_context(tc.tile_pool(name="mstream", bufs=3))
                if need_stream else None
            )
            mmps = ctx.enter_context(tc.tile_pool(name="mmps", bufs=2, space="PSUM"))
            spsp = ctx.enter_context(tc.tile_pool(name="sps", bufs=2, space="PSUM"))
            accp = ctx.enter_context(tc.tile_pool(name="acc", bufs=2, space="PSUM"))

            xq_r = xq.rearrange("(m p) s -> p m s", p=128)
            xk_r = xk.rearrange("(m p) s -> p m s", p=128)
            xv_r = xv.rearrange("(m p) s -> p m s", p=128)

            def dma_m2(out_tile, in_ap):
                # split the m (dim-1) axis into halves so dependents on the
                # first m-chunks unblock at half the transfer
                nc.sync.dma_start(out=out_tile[:, 0:4, :], in_=in_ap[:, 0:4, :])
                nc.sync.dma_start(out=out_tile[:, 4:8, :], in_=in_ap[:, 4:8, :])

            # --- critical-path DMAs first: the first sq column's x plus
            # the q/k weights, interleaved by m-halves so the projection
            # m-loops start as early as possible
            first_st = 0
            xq_t = xpool.tile([128, 8, 512], BF16, tag="xcol", name="xq_tc0")
            wq_sb = wpool.tile([128, 8, _DPC], BF16, tag="wq")
            xk_t = xpool.tile([128, 8, 512], BF16, tag="xcol", name="xk_tc0")
            wk_sb = wpool.tile([128, 8, _DPC], BF16, tag="wk")
            wv_sb = wpool.tile([128, 8, _DPC], BF16, tag="wv")
            wq_r = wq.rearrange("(m p) d -> p m d", p=128)
            wk_r = wk.rearrange("(m p) d -> p m d", p=128)
            wv_r = wv.rearrange("(m p) d -> p m d", p=128)
            for lo, hi in ((0, 4), (4, 8)):
                nc.sync.dma_start(out=xq_t[:, lo:hi, :],
                                  in_=xq_r[:, lo:hi, ts(first_st, 512)])
                nc.sync.dma_start(out=wq_sb[:, lo:hi, :], in_=wq_r[:, lo:hi, :])
                nc.sync.dma_start(out=xk_t[:, lo:hi, :],
                                  in_=xk_r[:, lo:hi, ts(first_st, 512)])
                nc.sync.dma_start(out=wk_sb[:, lo:hi, :], in_=wk_r[:, lo:hi, :])
            dma_m2(wv_sb, wv_r)

            bq_sb = cpool.tile([128, 2], F32, tag="bq")
            nc.sync.dma_start(out=bq_sb, in_=bq.rearrange("(h p) -> p h", p=128))
            bk_sb = cpool.tile([128, 2], F32, tag="bk")
            nc.sync.dma_start(out=bk_sb, in_=bk.rearrange("(h p) -> p h", p=128))
            bvb_sb = cpool.tile([128, _DPC], F32, tag="bvb")
            nc.sync.dma_start(out=bvb_sb, in_=bvb)
            if resident_mask:
                mask_sb = cpool.tile([128, nblk, 512], F32, tag="mask")
                nc.sync.dma_start(
                    out=mask_sb, in_=mblk.rearrange("n p s -> p n s")
                )
            wo_sb = wpool.tile([128, 2, _D], F32R, tag="wo")
            nc.sync.dma_start(out=wo_sb, in_=wo.rearrange("(c p) o -> p c o", p=128))

            aff_params = sorted({p for row in plan for (_, m, p) in row
                                 if m == 1})
            use_m01 = 0 < len(aff_params) <= 4
            if use_m01:
                m01 = cpool.tile([128, len(aff_params), 512], F32, tag="m01")
                nc.vector.memset(m01, 1.0)
                for oi, bp in enumerate(aff_params):
                    nc.gpsimd.affine_select(
                        out=m01[:, oi, :], in_=m01[:, oi, :],
                        compare_op=ALU.is_ge, fill=0.0, base=bp,
                        channel_multiplier=-1, pattern=[[1, 512]],
                    )

            ones32 = cpool.tile([1, 64], F32, tag="ones32")
            nc.vector.memset(ones32, 1.0)
            ones_r = cpool.tile([1, 64], F32R, tag="ones_r")
            nc.vector.tensor_copy(ones_r, ones32)
            onecol = cpool.tile([128, 1], F32, tag="onecol")
            nc.vector.memset(onecol, 1.0)

            # --- big SBUF state ---
            qT = biga.tile([128, 2, _S], F32R, tag="qT")
            kT = biga.tile([128, 2, _S], F32R, tag="kT")
            vsb = biga.tile([128, 16, _HPC * 65], F32R, tag="v")
            attn = biga.tile([128, 2, _S], F32R, tag="attn")

            # ones columns of v (softmax denominator trick)
            for sc in range(16):
                for h in range(_HPC):
                    nc.vector.tensor_copy(
                        vsb[:, sc, h * 65 + 64:h * 65 + 65], onecol
                    )

            # v-projection emitted lazily per 512-wide k-column group, the
            # first time any PV needs a chunk from it
            v_pending = set(range(4))

            def ensure_vgroup(col):
                if col not in v_pending:
                    return
                v_pending.discard(col)
                xv_t = xpool.tile([128, 8, 512], BF16, tag="xcol",
                                  name=f"xv_t{col}")
                dma_m2(xv_t, xv_r[:, :, ts(col, 512)])
                for c in range(4):
                    vps = mmps.tile([128, 512], F32, tag="mm", name="vps")
                    for m in range(8):
                        nc.tensor.matmul(
                            vps[:, 0:_DPC], lhsT=xv_t[:, m, ts(c, 128)],
                            rhs=wv_sb[:, m, :], start=(m == 0), stop=(m == 7),
                        )
                    sc = col * 4 + c
                    nc.vector.tensor_add(
                        vsb[:, sc, 0:260].rearrange(
                            "p (h x) -> p h x", x=65)[:, :, 0:64],
                        vps[:, 0:_DPC].rearrange("p (h x) -> p h x", x=64),
                        bvb_sb.rearrange("p (h x) -> p h x", x=64),
                    )

            # --- fused pipeline over sq columns (ascending: attention at
            # column i needs kT/v for all k-chunks <= i)
            def emit_outproj(st):
                # output projection for sq column st (row-sharded partial)
                for oc in range(8):
                    yps = accp.tile([128, 512], F32, tag="acc", name="yps")
                    for cc in range(2):
                        nc.tensor.matmul(
                            yps, lhsT=wo_sb[:, cc, ts(oc, 128)],
                            rhs=attn[:, cc, ts(st, 512)],
                            start=(cc == 0), stop=(cc == 1),
                        )
                    y_sb = yp.tile([128, 512], F32, tag="y", name="y_sb")
                    nc.vector.tensor_copy(y_sb, yps)
                    nc.sync.dma_start(
                        out=y[oc * 128:(oc + 1) * 128, ts(st, 512)], in_=y_sb
                    )

            for idx, st in enumerate((0, 1, 2, 3)):
                if idx > 0:
                    xq_t = xpool.tile([128, 8, 512], BF16, tag="xcol",
                                      name=f"xq_t{st}")
                    dma_m2(xq_t, xq_r[:, :, ts(st, 512)])
                    xk_t = xpool.tile([128, 8, 512], BF16, tag="xcol",
                                      name=f"xk_t{st}")
                    dma_m2(xk_t, xk_r[:, :, ts(st, 512)])

                # q/k projections for this column of sq
                for dh in range(2):
                    qps = mmps.tile([128, 512], F32, tag="mm", name="qps")
                    for m in range(8):
                        nc.tensor.matmul(
                            qps, lhsT=wq_sb[:, m, ts(dh, 128)], rhs=xq_t[:, m, :],
                            start=(m == 0), stop=(m == 7),
                        )
                    nc.vector.tensor_scalar(
                        qT[:, dh, ts(st, 512)], qps, bq_sb[:, dh:dh + 1], None,
                        ALU.add,
                    )
                    kps = mmps.tile([128, 512], F32, tag="mm", name="kps")
                    for m in range(8):
                        nc.tensor.matmul(
                            kps, lhsT=wk_sb[:, m, ts(dh, 128)], rhs=xk_t[:, m, :],
                            start=(m == 0), stop=(m == 7),
                        )
                    nc.vector.tensor_scalar(
                        kT[:, dh, ts(st, 512)], kps, bk_sb[:, dh:dh + 1], None,
                        ALU.add,
                    )

                # attention for sq tile i = st, both head pairs
                i = st
                blocks = plan[i]
                nj = len(blocks)
                for g in range(2):
                    acc = [
                        accp.tile([65, 512], F32, tag="acc", name=f"acc{g}{hh}")
                        for hh in range(2)
                    ]
                    for bi, (j, mode, param) in enumerate(blocks):
                        ensure_vgroup(j // 4)
                        sps = spsp.tile([128, 2, 512], F32, tag="sps", name="sps")
                        for hh in range(2):
                            nc.tensor.matmul(
                                sps[:, hh, :],
                                lhsT=kT[hh * 64:(hh + 1) * 64, g, ts(j, 128)],
                                rhs=qT[hh * 64:(hh + 1) * 64, g, ts(i, 512)],
                                start=True, stop=True,
                            )
                        if mode == 2:
                            if resident_mask:
                                mt = mask_sb[:, param, :]
                            else:
                                mt = mpool.tile([128, 512], F32, tag="mtile",
                                                name="mt")
                                nc.sync.dma_start(out=mt, in_=mblk[param])
                            for hh in range(2):
                                nc.vector.tensor_add(
                                    sps[:, hh, :], sps[:, hh, :], mt
                                )
                        probs = probp.tile([128, 2, 512], F32R, tag="probs",
                                           name="probs")
                        nc.scalar.activation(probs, sps, AF.Exp)
                        if mode == 1:
                            # masked cells satisfy s < p - base, p <= 127:
                            # only the first (128 - base) columns can be hit
                            ncols = min(512, 128 - param)
                            if ncols > 0 and use_m01:
                                oi = aff_params.index(param)
                                for hh in range(2):
                                    nc.vector.tensor_mul(
                                        probs[:, hh, 0:ncols],
                                        probs[:, hh, 0:ncols],
                                        m01[:, oi, 0:ncols],
                                    )
                            elif ncols > 0:
                                nc.gpsimd.affine_select(
                                    out=probs[:, :, 0:ncols],
                                    in_=probs[:, :, 0:ncols],
                                    compare_op=ALU.is_ge, fill=0.0,
                                    base=param, channel_multiplier=-1,
                                    pattern=[[0, 2], [1, ncols]],
                                )
                        for hh in range(2):
                            h = 2 * g + hh
                            nc.tensor.matmul(
                                acc[hh], lhsT=vsb[:, j, h * 65:(h + 1) * 65],
                                rhs=probs[:, hh, :],
                                start=(bi == 0), stop=(bi == nj - 1),
                            )
                    for hh in range(2):
                        rec = recp.tile([1, 512], F32R, tag="rec", name="rec")
                        with nc.allow_low_precision(
                            reason="softmax reciprocal; f32r storage"
                        ):
                            nc.vector.reciprocal(rec, acc[hh][64:65, :])
                        bc_ps = mmps.tile([64, 512], F32, tag="mm", name="bc_ps")
                        nc.tensor.matmul(bc_ps, lhsT=ones_r, rhs=rec)
                        bc_sb = bcp.tile([64, 512], F32, tag="bc", name="bc_sb")
                        nc.vector.tensor_copy(bc_sb, bc_ps)
                        nc.vector.tensor_mul(
                            attn[hh * 64:(hh + 1) * 64, g, ts(i, 512)],
                            acc[hh][0:64, :], bc_sb,
                        )

                emit_outproj(st)

    nc.compile()
    return nc


def kernel(**inputs):
    global LAST_RESULTS
    from concourse.bass_utils import run_bass_kernel_spmd

    Q = np.asarray(inputs["Q"], dtype=np.float32)
    K = np.asarray(inputs["K"], dtype=np.float32)
    V = np.asarray(inputs["V"], dtype=np.float32)
    mask = np.asarray(inputs["mask"], dtype=np.float32)
    Wq = np.asarray(inputs["Wq"], dtype=np.float32)
    bq = np.asarray(inputs["bq"], dtype=np.float32)
    Wk = np.asarray(inputs["Wk"], dtype=np.float32)
    bk = np.asarray(inputs["bk"], dtype=np.float32)
    Wv = np.asarray(inputs["Wv"], dtype=np.float32)
    bv = np.asarray(inputs["bv"], dtype=np.float32)
    Wo = np.asarray(inputs["Wo"], dtype=np.float32)
    bo = np.asarray(inputs["bo"], dtype=np.float32)

    plan, dense = _analyze_mask(mask)
    key = (plan, dense.shape[0])
    if key not in _program_cache:
        _program_cache[key] = _build_program(plan, dense.shape[0])
    nc = _program_cache[key]

    import ml_dtypes
    bf16 = ml_dtypes.bfloat16
    sc = np.float32(1.0 / np.sqrt(_DK))
    xqT = [np.ascontiguousarray(Q[b].T).astype(bf16) for b in range(_B)]
    xkT = [np.ascontiguousarray(K[b].T).astype(bf16) for b in range(_B)]
    xvT = [np.ascontiguousarray(V[b].T).astype(bf16) for b in range(_B)]

    in_maps = []
    for core in range(_NCORES):
        b = core // _CPG
        rows = slice((core % _CPG) * _DPC, (core % _CPG) * _DPC + _DPC)
        in_maps.append({
            "xq": xqT[b], "xk": xkT[b], "xv": xvT[b],
            "wq": np.ascontiguousarray((Wq[rows] * sc).T).astype(bf16),
            "wk": np.ascontiguousarray(Wk[rows].T).astype(bf16),
            "wv": np.ascontiguousarray(Wv[rows].T).astype(bf16),
            "wo": np.ascontiguousarray(Wo[:, rows].T),
            "bq": np.ascontiguousarray(bq[rows] * sc),
            "bk": np.ascontiguousarray(bk[rows]),
            "bvb": np.broadcast_to(bv[rows], (128, _DPC)).copy(),
            "mblk": dense,
        })

    trace = bool(int(os.environ.get("KERNEL_TRACE", "0")))
    LAST_RESULTS = run_bass_kernel_spmd(
        nc, in_maps, list(range(_NCORES)), trace=trace
    )

    out = np.empty((_B, _S, _D), np.float32)
    for b in range(_B):
        acc = np.zeros((_D, _S), np.float64)
        for c in range(_CPG):
            acc += LAST_RESULTS.results[b * _CPG + c]["y"]
        out[b] = (acc.T + bo.astype(np.float64)).astype(np.float32)
    return out

